# revision 10
# baseline (speedup 1.0000x reference)
"""Trainium2 Bass kernel for the sememe-GRU problem.

Math (per core, batch shard of BL=8):
    x = concat(inputs, sememe_h)                       # [T, BL, 1024]
    iou_x = x @ W_ioux.T + b_ioux + b_iouh             # precomputed on device
    fx_x  = x @ W_fx.T  + b_fx  + b_Uh                 # precomputed on device
    loop t:  iou = iou_x[t] + h @ W_iouh.T
             z, r = sigmoid(split(iou))
             h~ = tanh(fx_x[t] + (r*h) @ W_Uh.T)
             h  = (1-z)*h + z*h~  =  h~ + sigmoid(-iou_z)*(h - h~)

Layout: feature-major ("transposed") on chip — features on the 128 SBUF
partitions, (time, batch) on the free axis.  Recurrent matmuls keep the
weights stationary in the PE array (lhsT = W.T tile) and stream the tiny
[128, 8] h tile as the moving operand, so elementwise work runs on all 128
lanes with a free-dim of only 32.

Sharding: data-parallel over batch. 8 cores x BL=8 rows, weights replicated,
zero inter-core communication; the host casts/transposes weights into the
on-chip layouts, splits inputs, and regathers outputs.
"""

import numpy as np
import ml_dtypes

T, B, NINP, NHID = 256, 64, 512, 512
NCORES = 8
BL = B // NCORES          # 8 batch rows per core
KX = 2 * NINP             # 1024 concat features
G = 2 * NHID              # 1024 iou gates (z: 0..511, r: 512..1023)
NKX = KX // 128           # 8
NKH = NHID // 128         # 4
NMG = G // 128            # 8
NMH = NHID // 128         # 4
ROWS = T * BL             # 2048

BF16 = ml_dtypes.bfloat16
_CACHE = {}


def _build_nc(repeat=0):
    from contextlib import nullcontext
    from concourse import bacc, tile
    import concourse.mybir as mybir

    f32 = mybir.dt.float32
    bf16 = mybir.dt.bfloat16
    AF = mybir.ActivationFunctionType

    nc = bacc.Bacc("TRN2", target_bir_lowering=False, debug=False,
                   num_devices=NCORES)

    xT_p = nc.declare_dram_parameter("xT16", [128, NKX, ROWS], bf16, False)
    h0_p = nc.declare_dram_parameter("h0", [128, NMH, BL], f32, False)
    WxT_p = nc.declare_dram_parameter("WxT16", [128, NKX, G], bf16, False)
    WfxT_p = nc.declare_dram_parameter("WfxT16", [128, NKX, NHID], bf16, False)
    WhT_p = nc.declare_dram_parameter("WhT16", [128, NKH, G], bf16, False)
    WUT_p = nc.declare_dram_parameter("WUT16", [128, NKH, NHID], bf16, False)
    biou_p = nc.declare_dram_parameter("bias_iou", [128, NMG], f32, False)
    bu_p = nc.declare_dram_parameter("bias_u", [128, NMH], f32, False)
    # feature-major output: out_p[p, t, m, b] = h_{t+1}[b, m*128+p]
    out_p = nc.declare_dram_parameter("outT", [128, T, NMH, BL], bf16, True)

    with tile.TileContext(nc) as tc:
        with (
            tc.tile_pool(name="pers", bufs=1) as pers,
            tc.tile_pool(name="psum", bufs=2, space="PSUM") as psl,
            tc.tile_pool(name="xw", bufs=1) as xw,
            tc.tile_pool(name="work", bufs=3) as work,
            (tc.For_i(0, repeat, 1) if repeat else nullcontext()),
        ):
            WhT = pers.tile([128, NKH, G], bf16)
            WUT = pers.tile([128, NKH, NHID], bf16)
            iouxT = pers.tile([128, T, NMG, BL], f32)
            fxT = pers.tile([128, T, NMH, BL], f32)
            houtT = pers.tile([128, T + 1, NMH, BL], bf16)
            bias_iou = pers.tile([128, NMG], f32)
            bias_u = pers.tile([128, NMH], f32)

            # ---------------- phase A: loads ----------------
            if True:
                xT16 = xw.tile([128, NKX, ROWS], bf16)
                WxT = xw.tile([128, NKX, G], bf16)
                WfxT = xw.tile([128, NKX, NHID], bf16)

                nc.sync.dma_start(WxT[:], WxT_p[:, :, :])
                nc.sync.dma_start(WfxT[:], WfxT_p[:, :, :])
                nc.sync.dma_start(WhT[:], WhT_p[:, :, :])
                nc.sync.dma_start(WUT[:], WUT_p[:, :, :])
                nc.sync.dma_start(bias_iou[:], biou_p[:, :])
                nc.sync.dma_start(bias_u[:], bu_p[:, :])
                nc.sync.dma_start(xT16[:], xT_p[:, :, :])

                h0 = xw.tile([128, NMH, BL], f32)
                nc.sync.dma_start(h0[:], h0_p[:, :, :])
                nc.vector.tensor_copy(houtT[:, 0, :, :], h0[:])

                # -------------- phase B: input projections --------------
                NRC = 4                  # row chunks of 512
                RC = ROWS // NRC         # 512 rows per chunk = 64 t x 8 b
                TC = RC // BL            # 64 timesteps per chunk
                for m in range(NMG):
                    for n in range(NRC):
                        pt = psl.tile([128, TC, BL], f32, tag="ppre")
                        for k in range(NKX):
                            nc.tensor.matmul(
                                pt[:, :, :],
                                lhsT=WxT[:, k, m * 128:(m + 1) * 128],
                                rhs=xT16[:, k, n * RC:(n + 1) * RC],
                                start=(k == 0), stop=(k == NKX - 1))
                        nc.scalar.activation(
                            iouxT[:, n * TC:(n + 1) * TC, m, :], pt[:, :, :],
                            AF.Identity, bias=bias_iou[:, m:m + 1])
                for m in range(NMH):
                    for n in range(NRC):
                        pt = psl.tile([128, TC, BL], f32, tag="ppre")
                        for k in range(NKX):
                            nc.tensor.matmul(
                                pt[:, :, :],
                                lhsT=WfxT[:, k, m * 128:(m + 1) * 128],
                                rhs=xT16[:, k, n * RC:(n + 1) * RC],
                                start=(k == 0), stop=(k == NKX - 1))
                        nc.scalar.activation(
                            fxT[:, n * TC:(n + 1) * TC, m, :], pt[:, :, :],
                            AF.Identity, bias=bias_u[:, m:m + 1])

            # ---------------- phase C: the recurrence ----------------
            if True:
                for s in range(T):
                    hprev = houtT[:, s, :, :]
                    ps_r = psl.tile([128, NKH, BL], f32, tag="ps_r")
                    ps_z = psl.tile([128, NKH, BL], f32, tag="ps_z")
                    ps_u = psl.tile([128, NKH, BL], f32, tag="ps_u")
                    # MM1 r-half (gates 512..1023 = m 4..7)
                    for mi, m in enumerate(range(NMH, NMG)):
                        for k in range(NKH):
                            nc.tensor.matmul(
                                ps_r[:, mi, :],
                                lhsT=WhT[:, k, m * 128:(m + 1) * 128],
                                rhs=hprev[:, k, :],
                                start=(k == 0), stop=(k == NKH - 1))
                    # MM1 z-half (gates 0..511)
                    for m in range(NMH):
                        for k in range(NKH):
                            nc.tensor.matmul(
                                ps_z[:, m, :],
                                lhsT=WhT[:, k, m * 128:(m + 1) * 128],
                                rhs=hprev[:, k, :],
                                start=(k == 0), stop=(k == NKH - 1))
                    # r = sigmoid(ps_r + ioux_r);  rh = r * h
                    rsum = work.tile([128, NKH, BL], f32, tag="rsum")
                    nc.vector.tensor_add(
                        rsum[:], ps_r[:, :, :], iouxT[:, s, NMH:NMG, :])
                    r16 = work.tile([128, NKH, BL], bf16, tag="r16")
                    nc.scalar.activation(r16[:], rsum[:], AF.Sigmoid)
                    rh16 = work.tile([128, NKH, BL], bf16, tag="rh16")
                    nc.vector.tensor_mul(rh16[:], r16[:], hprev)
                    # MM2: u = W_Uh @ rh
                    for m in range(NMH):
                        for k in range(NKH):
                            nc.tensor.matmul(
                                ps_u[:, m, :],
                                lhsT=WUT[:, k, m * 128:(m + 1) * 128],
                                rhs=rh16[:, k, :],
                                start=(k == 0), stop=(k == NKH - 1))
                    # zc = sigmoid(-(ps_z + ioux_z)) = 1 - z
                    zsum = work.tile([128, NKH, BL], f32, tag="zsum")
                    nc.vector.tensor_add(
                        zsum[:], ps_z[:, :, :], iouxT[:, s, 0:NMH, :])
                    zc16 = work.tile([128, NKH, BL], bf16, tag="zc16")
                    nc.scalar.activation(zc16[:], zsum[:], AF.Sigmoid,
                                         scale=-1.0)
                    # h~ = tanh(ps_u + fx)
                    usum = work.tile([128, NKH, BL], f32, tag="usum")
                    nc.vector.tensor_add(usum[:], ps_u[:, :, :], fxT[:, s, :, :])
                    ht16 = work.tile([128, NKH, BL], bf16, tag="ht16")
                    nc.scalar.activation(ht16[:], usum[:], AF.Tanh)
                    # h_new = h~ + zc * (h - h~)
                    d16 = work.tile([128, NKH, BL], bf16, tag="d16")
                    nc.vector.tensor_sub(d16[:], hprev, ht16[:])
                    e16 = work.tile([128, NKH, BL], bf16, tag="e16")
                    nc.vector.tensor_mul(e16[:], zc16[:], d16[:])
                    nc.vector.tensor_add(houtT[:, s + 1, :, :], ht16[:], e16[:])

                nc.sync.dma_start(out_p[:, :, :, :], houtT[:, 1:, :, :])

    nc.compile()
    return nc


def _get_nc(repeat=0):
    key = ("nc", repeat)
    if key not in _CACHE:
        _CACHE[key] = _build_nc(repeat)
    return _CACHE[key]


def _wT_layout(W, nk):
    """W [M, K] fp32 -> W.T in on-chip layout [128, nk, M] bf16."""
    M, K = W.shape
    assert nk * 128 == K
    return np.ascontiguousarray(
        W.T.reshape(nk, 128, M).transpose(1, 0, 2)).astype(BF16)


def make_in_maps(inputs, sememe_h, hx, W_ioux, b_ioux, W_iouh, b_iouh,
                 W_fx, b_fx, W_Uh, b_Uh):
    inputs = np.asarray(inputs, np.float32)
    sememe_h = np.asarray(sememe_h, np.float32)
    hx = np.asarray(hx, np.float32)
    WxT16 = _wT_layout(np.asarray(W_ioux, np.float32), NKX)
    WfxT16 = _wT_layout(np.asarray(W_fx, np.float32), NKX)
    WhT16 = _wT_layout(np.asarray(W_iouh, np.float32), NKH)
    WUT16 = _wT_layout(np.asarray(W_Uh, np.float32), NKH)
    bias_iou = np.ascontiguousarray(
        (np.asarray(b_ioux, np.float32) + np.asarray(b_iouh, np.float32))
        .reshape(NMG, 128).T)
    bias_u = np.ascontiguousarray(
        (np.asarray(b_fx, np.float32) + np.asarray(b_Uh, np.float32))
        .reshape(NMH, 128).T)
    shared = {
        "WxT16": WxT16, "WfxT16": WfxT16, "WhT16": WhT16, "WUT16": WUT16,
        "bias_iou": bias_iou, "bias_u": bias_u,
    }
    inp16 = inputs.astype(BF16)
    sem16 = sememe_h.astype(BF16)
    in_maps = []
    for c in range(NCORES):
        sl = slice(c * BL, (c + 1) * BL)
        # xT16[p, k, t*BL+b] = x[t, b, k*128+p]; k<NKX/2 -> inputs, else sememe
        xT16 = np.empty((128, NKX, ROWS), BF16)
        xT16[:, :NKX // 2, :] = (
            inp16[:, sl, :].reshape(ROWS, NKX // 2, 128).transpose(2, 1, 0))
        xT16[:, NKX // 2:, :] = (
            sem16[:, sl, :].reshape(ROWS, NKX // 2, 128).transpose(2, 1, 0))
        # h0[p, m, b] = hx[b, m*128+p]
        h0 = np.ascontiguousarray(
            hx[sl, :].reshape(BL, NMH, 128).transpose(2, 1, 0))
        in_maps.append({"xT16": xT16, "h0": h0, **shared})
    return in_maps


def kernel(inputs, sememe_h, hx, W_ioux, b_ioux, W_iouh, b_iouh,
           W_fx, b_fx, W_Uh, b_Uh):
    from concourse.bass_utils import run_bass_kernel_spmd

    nc = _get_nc()
    in_maps = make_in_maps(inputs, sememe_h, hx, W_ioux, b_ioux,
                           W_iouh, b_iouh, W_fx, b_fx, W_Uh, b_Uh)
    res = run_bass_kernel_spmd(nc, in_maps, list(range(NCORES)))
    outs = []
    for c in range(NCORES):
        o = np.asarray(res.results[c]["outT"]).astype(np.float32)
        # o[p, t, m, b] = h_t[b, m*128+p]  ->  [t, b, m, p] -> [T, BL, NHID]
        outs.append(o.transpose(1, 3, 2, 0).reshape(T, BL, NHID))
    out = np.concatenate(outs, axis=1)          # [T, B, NHID]
    return out, out[-1]


# revision 12
# speedup vs baseline: 68.3491x; 68.3491x over previous
"""Trainium2 Bass kernel for the sememe-GRU problem.

Math (per core, batch shard of BL=8):
    x = concat(inputs, sememe_h)                       # [T, BL, 1024]
    iou_x = x @ W_ioux.T + b_ioux + b_iouh             # precomputed on device
    fx_x  = x @ W_fx.T  + b_fx  + b_Uh                 # precomputed on device
    loop t:  iou = iou_x[t] + h @ W_iouh.T
             z, r = sigmoid(split(iou))
             h~ = tanh(fx_x[t] + (r*h) @ W_Uh.T)
             h  = (1-z)*h + z*h~  =  h~ + sigmoid(-iou_z)*(h - h~)

Layout: feature-major ("transposed") on chip — features on the 128 SBUF
partitions, (time, batch) on the free axis.  Recurrent matmuls keep the
weights stationary in the PE array (lhsT = W.T tile) and stream the tiny
[128, 8] h tile as the moving operand, so elementwise work runs on all 128
lanes with a free-dim of only 32.

Sharding: data-parallel over batch. 8 cores x BL=8 rows, weights replicated,
zero inter-core communication; the host casts/transposes weights into the
on-chip layouts, splits inputs, and regathers outputs.
"""

import numpy as np
import ml_dtypes

T, B, NINP, NHID = 256, 64, 512, 512
NCORES = 8
BL = B // NCORES          # 8 batch rows per core
KX = 2 * NINP             # 1024 concat features
G = 2 * NHID              # 1024 iou gates (z: 0..511, r: 512..1023)
NKX = KX // 128           # 8
NKH = NHID // 128         # 4
NMG = G // 128            # 8
NMH = NHID // 128         # 4
ROWS = T * BL             # 2048

BF16 = ml_dtypes.bfloat16
_CACHE = {}


def _build_nc(repeat=0, no_precompute=False, small_chain=False):
    from contextlib import nullcontext
    from concourse import bacc, tile
    import concourse.mybir as mybir

    f32 = mybir.dt.float32
    bf16 = mybir.dt.bfloat16
    AF = mybir.ActivationFunctionType

    nc = bacc.Bacc("TRN2", target_bir_lowering=False, debug=False,
                   num_devices=NCORES)

    xT_p = nc.declare_dram_parameter("xT16", [128, NKX, ROWS], bf16, False)
    h0_p = nc.declare_dram_parameter("h0", [128, NMH, BL], f32, False)
    WxT_p = nc.declare_dram_parameter("WxT16", [128, NKX, G], bf16, False)
    WfxT_p = nc.declare_dram_parameter("WfxT16", [128, NKX, NHID], bf16, False)
    WhT_p = nc.declare_dram_parameter("WhT16", [128, NKH, G], bf16, False)
    WUT_p = nc.declare_dram_parameter("WUT16", [128, NKH, NHID], bf16, False)
    biou_p = nc.declare_dram_parameter("bias_iou", [128, NMG], f32, False)
    bu_p = nc.declare_dram_parameter("bias_u", [128, NMH], f32, False)
    # feature-major output: out_p[p, t, m, b] = h_{t+1}[b, m*128+p]
    out_p = nc.declare_dram_parameter("outT", [128, T, NMH, BL], bf16, True)

    with tile.TileContext(nc) as tc:
        with (
            tc.tile_pool(name="pers", bufs=1) as pers,
            tc.tile_pool(name="psum", bufs=2, space="PSUM") as psl,
            tc.tile_pool(name="xw", bufs=1) as xw,
            tc.tile_pool(name="work", bufs=3) as work,
            (tc.For_i(0, repeat, 1) if repeat else nullcontext()),
        ):
            WhT = pers.tile([128, NKH, G], bf16)
            WUT = pers.tile([128, NKH, NHID], bf16)
            iouxT = pers.tile([128, T, NMG, BL], f32)
            fxT = pers.tile([128, T, NMH, BL], f32)
            houtT = pers.tile([128, T + 1, NMH, BL], bf16)
            bias_iou = pers.tile([128, NMG], f32)
            bias_u = pers.tile([128, NMH], f32)

            # ---------------- phase A: loads ----------------
            if True:
                xT16 = xw.tile([128, NKX, ROWS], bf16)
                WxT = xw.tile([128, NKX, G], bf16)
                WfxT = xw.tile([128, NKX, NHID], bf16)

                nc.sync.dma_start(WxT[:], WxT_p[:, :, :])
                nc.sync.dma_start(WfxT[:], WfxT_p[:, :, :])
                nc.sync.dma_start(WhT[:], WhT_p[:, :, :])
                nc.sync.dma_start(WUT[:], WUT_p[:, :, :])
                nc.sync.dma_start(bias_iou[:], biou_p[:, :])
                nc.sync.dma_start(bias_u[:], bu_p[:, :])
                nc.sync.dma_start(xT16[:], xT_p[:, :, :])

                h0 = xw.tile([128, NMH, BL], f32)
                nc.sync.dma_start(h0[:], h0_p[:, :, :])
                nc.vector.tensor_copy(houtT[:, 0, :, :], h0[:])

                # -------------- phase B: input projections --------------
                NRC = 4                  # row chunks of 512
                RC = ROWS // NRC         # 512 rows per chunk = 64 t x 8 b
                TC = RC // BL            # 64 timesteps per chunk
                if no_precompute:
                    nc.vector.memset(iouxT[:], 0.01)
                    nc.vector.memset(fxT[:], 0.01)

                def precompute_chunk(n):
                    if no_precompute:
                        return
                    for m in range(NMG):
                        pt = psl.tile([128, TC, BL], f32, tag="ppre")
                        for k in range(NKX):
                            nc.tensor.matmul(
                                pt[:, :, :],
                                lhsT=WxT[:, k, m * 128:(m + 1) * 128],
                                rhs=xT16[:, k, n * RC:(n + 1) * RC],
                                start=(k == 0), stop=(k == NKX - 1))
                        nc.scalar.activation(
                            iouxT[:, n * TC:(n + 1) * TC, m, :], pt[:, :, :],
                            AF.Identity, bias=bias_iou[:, m:m + 1])
                    for m in range(NMH):
                        pt = psl.tile([128, TC, BL], f32, tag="ppre")
                        for k in range(NKX):
                            nc.tensor.matmul(
                                pt[:, :, :],
                                lhsT=WfxT[:, k, m * 128:(m + 1) * 128],
                                rhs=xT16[:, k, n * RC:(n + 1) * RC],
                                start=(k == 0), stop=(k == NKX - 1))
                        nc.scalar.activation(
                            fxT[:, n * TC:(n + 1) * TC, m, :], pt[:, :, :],
                            AF.Identity, bias=bias_u[:, m:m + 1])

                precompute_chunk(0)

            # ---------------- phase C: the recurrence ----------------
            if True:
                hprev = houtT[:, 0, :, :]
                for s in range(T):
                    if s % TC == 0 and s // TC + 1 < NRC:
                        precompute_chunk(s // TC + 1)
                    if not small_chain:
                        hprev = houtT[:, s, :, :]
                    ps_r = psl.tile([128, NKH, BL], f32, tag="ps_r")
                    ps_z = psl.tile([128, NKH, BL], f32, tag="ps_z")
                    ps_u = psl.tile([128, NKH, BL], f32, tag="ps_u")
                    # MM1 r-half (gates 512..1023 = m 4..7)
                    for mi, m in enumerate(range(NMH, NMG)):
                        for k in range(NKH):
                            nc.tensor.matmul(
                                ps_r[:, mi, :],
                                lhsT=WhT[:, k, m * 128:(m + 1) * 128],
                                rhs=hprev[:, k, :],
                                start=(k == 0), stop=(k == NKH - 1))
                    # MM1 z-half (gates 0..511)
                    for m in range(NMH):
                        for k in range(NKH):
                            nc.tensor.matmul(
                                ps_z[:, m, :],
                                lhsT=WhT[:, k, m * 128:(m + 1) * 128],
                                rhs=hprev[:, k, :],
                                start=(k == 0), stop=(k == NKH - 1))
                    # r = sigmoid(ps_r + ioux_r);  rh = r * h
                    rsum = work.tile([128, NKH, BL], f32, tag="rsum")
                    nc.vector.tensor_add(
                        rsum[:], ps_r[:, :, :], iouxT[:, s, NMH:NMG, :])
                    r16 = work.tile([128, NKH, BL], bf16, tag="r16")
                    nc.scalar.activation(r16[:], rsum[:], AF.Sigmoid)
                    rh16 = work.tile([128, NKH, BL], bf16, tag="rh16")
                    nc.vector.tensor_mul(rh16[:], r16[:], hprev)
                    # MM2: u = W_Uh @ rh
                    for m in range(NMH):
                        for k in range(NKH):
                            nc.tensor.matmul(
                                ps_u[:, m, :],
                                lhsT=WUT[:, k, m * 128:(m + 1) * 128],
                                rhs=rh16[:, k, :],
                                start=(k == 0), stop=(k == NKH - 1))
                    # zc = sigmoid(-(ps_z + ioux_z)) = 1 - z
                    zsum = work.tile([128, NKH, BL], f32, tag="zsum")
                    nc.vector.tensor_add(
                        zsum[:], ps_z[:, :, :], iouxT[:, s, 0:NMH, :])
                    zc16 = work.tile([128, NKH, BL], bf16, tag="zc16")
                    nc.scalar.activation(zc16[:], zsum[:], AF.Sigmoid,
                                         scale=-1.0)
                    # h~ = tanh(ps_u + fx)
                    usum = work.tile([128, NKH, BL], f32, tag="usum")
                    nc.vector.tensor_add(usum[:], ps_u[:, :, :], fxT[:, s, :, :])
                    ht16 = work.tile([128, NKH, BL], bf16, tag="ht16")
                    nc.scalar.activation(ht16[:], usum[:], AF.Tanh)
                    # h_new = h~ + zc * (h - h~)
                    d16 = work.tile([128, NKH, BL], bf16, tag="d16")
                    nc.vector.tensor_sub(d16[:], hprev, ht16[:])
                    e16 = work.tile([128, NKH, BL], bf16, tag="e16")
                    nc.vector.tensor_mul(e16[:], zc16[:], d16[:])
                    if small_chain:
                        hn = work.tile([128, NKH, BL], bf16, tag="hn")
                        nc.vector.tensor_add(hn[:], ht16[:], e16[:])
                        nc.scalar.copy(houtT[:, s + 1, :, :], hn[:])
                        hprev = hn[:, :, :]
                    else:
                        nc.vector.tensor_add(houtT[:, s + 1, :, :], ht16[:], e16[:])

                nc.sync.dma_start(out_p[:, :, :, :], houtT[:, 1:, :, :])

    nc.compile()
    return nc


def _get_nc(repeat=0, **kw):
    key = ("nc", repeat, tuple(sorted(kw.items())))
    if key not in _CACHE:
        _CACHE[key] = _build_nc(repeat, **kw)
    return _CACHE[key]


def _wT_layout(W, nk):
    """W [M, K] fp32 -> W.T in on-chip layout [128, nk, M] bf16."""
    M, K = W.shape
    assert nk * 128 == K
    return np.ascontiguousarray(
        W.T.reshape(nk, 128, M).transpose(1, 0, 2)).astype(BF16)


def make_in_maps(inputs, sememe_h, hx, W_ioux, b_ioux, W_iouh, b_iouh,
                 W_fx, b_fx, W_Uh, b_Uh):
    inputs = np.asarray(inputs, np.float32)
    sememe_h = np.asarray(sememe_h, np.float32)
    hx = np.asarray(hx, np.float32)
    WxT16 = _wT_layout(np.asarray(W_ioux, np.float32), NKX)
    WfxT16 = _wT_layout(np.asarray(W_fx, np.float32), NKX)
    WhT16 = _wT_layout(np.asarray(W_iouh, np.float32), NKH)
    WUT16 = _wT_layout(np.asarray(W_Uh, np.float32), NKH)
    bias_iou = np.ascontiguousarray(
        (np.asarray(b_ioux, np.float32) + np.asarray(b_iouh, np.float32))
        .reshape(NMG, 128).T)
    bias_u = np.ascontiguousarray(
        (np.asarray(b_fx, np.float32) + np.asarray(b_Uh, np.float32))
        .reshape(NMH, 128).T)
    shared = {
        "WxT16": WxT16, "WfxT16": WfxT16, "WhT16": WhT16, "WUT16": WUT16,
        "bias_iou": bias_iou, "bias_u": bias_u,
    }
    inp16 = inputs.astype(BF16)
    sem16 = sememe_h.astype(BF16)
    in_maps = []
    for c in range(NCORES):
        sl = slice(c * BL, (c + 1) * BL)
        # xT16[p, k, t*BL+b] = x[t, b, k*128+p]; k<NKX/2 -> inputs, else sememe
        xT16 = np.empty((128, NKX, ROWS), BF16)
        xT16[:, :NKX // 2, :] = (
            inp16[:, sl, :].reshape(ROWS, NKX // 2, 128).transpose(2, 1, 0))
        xT16[:, NKX // 2:, :] = (
            sem16[:, sl, :].reshape(ROWS, NKX // 2, 128).transpose(2, 1, 0))
        # h0[p, m, b] = hx[b, m*128+p]
        h0 = np.ascontiguousarray(
            hx[sl, :].reshape(BL, NMH, 128).transpose(2, 1, 0))
        in_maps.append({"xT16": xT16, "h0": h0, **shared})
    return in_maps


def kernel(inputs, sememe_h, hx, W_ioux, b_ioux, W_iouh, b_iouh,
           W_fx, b_fx, W_Uh, b_Uh):
    from concourse.bass_utils import run_bass_kernel_spmd

    nc = _get_nc()
    in_maps = make_in_maps(inputs, sememe_h, hx, W_ioux, b_ioux,
                           W_iouh, b_iouh, W_fx, b_fx, W_Uh, b_Uh)
    res = run_bass_kernel_spmd(nc, in_maps, list(range(NCORES)))
    outs = []
    for c in range(NCORES):
        o = np.asarray(res.results[c]["outT"]).astype(np.float32)
        # o[p, t, m, b] = h_t[b, m*128+p]  ->  [t, b, m, p] -> [T, BL, NHID]
        outs.append(o.transpose(1, 3, 2, 0).reshape(T, BL, NHID))
    out = np.concatenate(outs, axis=1)          # [T, B, NHID]
    return out, out[-1]
